# revision 50
# baseline (speedup 1.0000x reference)
"""RWKV v4 block (nn_Block_15109694947416) on 8 TRN2 NeuronCores.

Strategy (v3, fused + matmul-folded mixes):
- Data-parallel over B: core i processes batch i. No collectives.
- Two fused passes in one TileContext: pass A = LN1 + time-mix + WKV + Wo +
  residual (x2 in a DRAM scratch), pass B = LN2 + FFN + residual.
- LN gains/biases are folded into the weights host-side; bias rows (b@W) are
  added via the ACT bias port at PSUM-evacuation time.
- Time-mix token-shift interpolation is folded into the matmuls:
  k = s1 @ (diag(g*tmk) Wk) + s0 @ (diag(g*(1-tmk)) Wk) + b@Wk, where s is
  the normalized activation (fp8) and s0 its one-token shift — so no
  elementwise mix ops at all. All time-mix matmuls are fp8 DoubleRow
  (weights pre-scaled x256 out of e4m3 subnormal range; 1/256 folded into
  the ACT evacuation scale).
- sigmoid(z) computed as 0.5*(1+tanh(z/2)) so pass A's ACT functions all
  live in one activation-table set (no per-chunk table thrash); the 0.5 is
  folded into the v path / fWv weights.
- WKV runs unstabilized in fp32 via the hardware tensor_tensor_scan.
- Engine split respects the neuronxcc rule that Pool only runs
  TensorTensor/TensorCopy/Memset: scans+STT on DVE, big f32 multiplies on
  Pool, PSUM evacuation on ACT (+ some relu-STT on DVE).
- End-to-end rel-err budget measured in numpy: ~1.3e-2 vs the 2e-2 gate.
"""

import numpy as np
import ml_dtypes

B, T, C = 8, 2048, 1024
TC = 256                 # time chunk
NCH = T // TC            # 8 chunks
CB = C // 128            # 8 channel blocks
FB = 4 * C // 128        # 32 ffn hidden blocks
EPS = 1e-5

FWK_F8 = False           # fWk in fp8 DoubleRow (accuracy-risky)
WS = 256.0               # fp8 weight pre-scale (keeps w out of e4m3 subnormals)

_CACHE = {}


def _bcast_free(ap, n):
    """[128,1] AP -> [128,n] stride-0 broadcast along free dim."""
    import concourse.bass as bass
    return bass.AP(tensor=ap.tensor, offset=ap.offset, ap=[ap.ap[0], [0, n]])


def _bcast_last(ap, n):
    """[128,M,1] AP -> [128,M,n] stride-0 broadcast of the last dim."""
    import concourse.bass as bass
    return bass.AP(tensor=ap.tensor, offset=ap.offset,
                   ap=[ap.ap[0], ap.ap[1], [0, n]])


def _bcast_mid(ap, nmid):
    """[128,N] AP -> [128,nmid,N] stride-0 broadcast of a middle dim."""
    import concourse.bass as bass
    return bass.AP(tensor=ap.tensor, offset=ap.offset,
                   ap=[ap.ap[0], [0, nmid], ap.ap[1]])


def _build():
    import concourse.bass as bass
    import concourse.bacc as bacc
    import concourse.tile as tile
    from concourse import mybir
    import contextlib

    f32 = mybir.dt.float32
    f32r = mybir.dt.float32r
    bf16 = mybir.dt.bfloat16
    f8 = mybir.dt.float8e4
    AF = mybir.ActivationFunctionType
    OP = mybir.AluOpType
    DRM = mybir.MatmulPerfMode.DoubleRow

    fwk_dt = f8 if FWK_F8 else bf16

    nc = bacc.Bacc(None, target_bir_lowering=False, debug=False)

    xTf = nc.dram_tensor("xTf", [C, T], f32, kind="ExternalInput")
    xTb = nc.dram_tensor("xTb", [C, T], bf16, kind="ExternalInput")
    NROW = 8
    (EW, EU, BK, BV, BR, FTMK, FTMR, BFR) = range(NROW)
    cvecs = nc.dram_tensor("cvecs", [128, CB, NROW], f32, kind="ExternalInput")
    fbk = nc.dram_tensor("fbk", [128, FB], f32, kind="ExternalInput")
    oneC = nc.dram_tensor("oneC", [128], f32r, kind="ExternalInput")  # 1/C
    onePM = nc.dram_tensor("onePM", [2, 128], bf16, kind="ExternalInput")
    # doubled (mix-folded) time-mix weights, fp8, [2, C, C]: [0]=diag(tm)W,
    # [1]=diag(1-tm)W
    Wk = nc.dram_tensor("Wk", [2, C, C], f8, kind="ExternalInput")
    Wv = nc.dram_tensor("Wv", [2, C, C], f8, kind="ExternalInput")
    Wr = nc.dram_tensor("Wr", [2, C, C], f8, kind="ExternalInput")
    Wo = nc.dram_tensor("Wo", [C, C], f8, kind="ExternalInput")
    fWk = nc.dram_tensor("fWk", [C, 4 * C], fwk_dt, kind="ExternalInput")
    fWv = nc.dram_tensor("fWv", [4 * C, C], f8, kind="ExternalInput")
    fWr = nc.dram_tensor("fWr", [C, C], f8, kind="ExternalInput")
    outT = nc.dram_tensor("outT", [C, T], f32, kind="ExternalOutput")

    def ldw(wt, src, parts=4, eng=None):
        """Load weight [C_in, M] (or [2, C_in, M]) as [128, (2*)CB, M]."""
        eng = eng or nc.sync
        if len(src.shape) == 3:
            ap = src.rearrange("two (a p) m -> p (two a) m", p=128)
        else:
            ap = src.rearrange("(a p) m -> p a m", p=128)
        nblk = wt.shape[1]
        step = max(1, nblk // parts)
        for i in range(0, nblk, step):
            j = min(i + step, nblk)
            eng.dma_start(out=wt[:, i:j, :], in_=ap[:, i:j, :])

    def ldw_m(wt, src, mparts=8, eng=None):
        """Load weight [C_in, M] as [128, CB_in, M], split along M so the
        first output blocks become usable early."""
        eng = eng or nc.sync
        ap = src.rearrange("(a p) m -> p a m", p=128)
        M = wt.shape[2]
        step = M // mparts
        for i in range(0, M, step):
            j = min(i + step, M)
            eng.dma_start(out=wt[:, :, i:j], in_=ap[:, :, i:j])

    def dr_group(ps, w_t, in_t, csl, start=True, stop=True):
        """Accumulate ps[:, :] += sum_a w[:, a, csl].T @ in[:, a, :], fp8 DR."""
        nk = in_t.shape[1] // 2
        for i in range(nk):
            nc.tensor.matmul(ps, w_t[:, 2 * i:2 * i + 2, csl],
                             in_t[:, 2 * i:2 * i + 2, :],
                             start=start and (i == 0),
                             stop=stop and (i == nk - 1), perf_mode=DRM)

    def bf_group(ps, w_t, in_t, csl):
        nk = in_t.shape[1]
        for a in range(nk):
            nc.tensor.matmul(ps, w_t[:, a, csl], in_t[:, a, :],
                             start=(a == 0), stop=(a == nk - 1))

    with tile.TileContext(nc) as tc:
        with contextlib.ExitStack() as ctx:
            consts = ctx.enter_context(tc.tile_pool(name="consts", bufs=1))
            dramp = ctx.enter_context(tc.tile_pool(name="dram", bufs=1, space="DRAM"))
            wfr = ctx.enter_context(tc.tile_pool(name="wfr", bufs=1))

            cv = consts.tile([128, CB, NROW], f32)
            nc.sync.dma_start(out=cv, in_=cvecs[:, :, :])
            fbk_t = consts.tile([128, FB, 1], f32)
            nc.sync.dma_start(out=fbk_t, in_=fbk.rearrange("p (a o) -> p a o", o=1))
            oneC_f = consts.tile([128, 1], f32r)
            nc.sync.dma_start(out=oneC_f, in_=oneC.rearrange("(p o) -> p o", o=1))
            oneC_b = consts.tile([128, 1], bf16)
            nc.vector.tensor_copy(out=oneC_b, in_=oneC_f.bitcast(f32))
            one_pos = consts.tile([1, 128], bf16)
            nc.sync.dma_start(out=one_pos, in_=onePM[0:1, :])
            one_neg = consts.tile([1, 128], bf16)
            nc.sync.dma_start(out=one_neg, in_=onePM[1:2, :])
            eps_t = consts.tile([1, 1], f32)
            nc.vector.memset(eps_t, EPS)
            zeros_b = consts.tile([128, TC], bf16)
            nc.vector.memset(zeros_b, 0.0)
            one_row = consts.tile([128, 1], f32)
            nc.vector.memset(one_row, 1.0)
            invS_o = consts.tile([128, 1], f32)
            nc.vector.memset(invS_o, 1.0 / WS)
            eu_full = consts.tile([128, CB, TC], bf16)
            nc.vector.tensor_copy(out=eu_full,
                                  in_=_bcast_last(cv[:, :, EU:EU + 1], TC))
            sK = 1.0 / WS
            sV = sR = 0.5 / WS
            sFR = 0.5 / WS
            sFV = 1.0 / WS
            sFK = (1.0 / WS) if FWK_F8 else 1.0

            # fWr preloaded before pass A (SBUF is free then); fWk/fWv load
            # at the pass boundary.
            fwr_t = wfr.tile([128, CB, C], f8, tag="fwr")
            ldw(fwr_t, fWr, parts=2)

            x2d = dramp.tile([NCH, 128, CB, TC], f32r)

            def layer_norm(ps_stat, ps_bc, rows, rtmp, mrb, x_mm, sq_t):
                """mrb[:,0,:]=rstd, mrb[:,1,:]=-m*rstd (bf16) for one chunk."""
                st = ps_stat.tile([1, 2, TC], f32, tag="st")
                lhs_x = oneC_f if x_mm.dtype == f32r else oneC_b
                for cb in range(CB):
                    nc.tensor.matmul(st[:, 0, :], lhs_x, x_mm[:, cb, :],
                                     start=(cb == 0), stop=(cb == CB - 1))
                for cb in range(CB):
                    nc.tensor.matmul(st[:, 1, :], oneC_b, sq_t[:, cb, :],
                                     start=(cb == 0), stop=(cb == CB - 1))
                nc.scalar.activation(out=rtmp[:, 1, :], in_=st[:, 0, :],
                                     func=AF.Square)
                nc.vector.tensor_sub(rtmp[:, 1, :], st[:, 1, :], rtmp[:, 1, :])
                nc.scalar.activation(out=rtmp[:, 1, :], in_=rtmp[:, 1, :],
                                     func=AF.Sqrt, bias=eps_t[:, :])
                nc.vector.reciprocal_approx_fast(out=rtmp[:, 0, :],
                                                 in_=rtmp[:, 1, :])
                nc.vector.tensor_copy(out=rows[:, 0, :], in_=rtmp[:, 0, :])
                nc.vector.tensor_mul(rows[:, 1, :], st[:, 0, :], rtmp[:, 0, :])
                bc = ps_bc.tile([128, 2, TC], f32, tag="bc")
                nc.tensor.matmul(bc[:, 0, :], one_pos, rows[:, 0, :])
                nc.tensor.matmul(bc[:, 1, :], one_neg, rows[:, 1, :])
                nc.scalar.activation(out=mrb, in_=bc, func=AF.Copy)

            # ================= Pass A: time-mix =================
            with contextlib.ExitStack() as pA:
                wp = pA.enter_context(tc.tile_pool(name="wA", bufs=1))
                dbl = pA.enter_context(tc.tile_pool(name="dA", bufs=2))
                sgf = pA.enter_context(tc.tile_pool(name="sgf", bufs=1))
                sc1 = pA.enter_context(tc.tile_pool(name="scA", bufs=1))
                sgA = pA.enter_context(tc.tile_pool(name="sgA", bufs=2))
                ps_stat = pA.enter_context(tc.tile_pool(name="psA_st", bufs=1, space="PSUM"))
                ps_bc = pA.enter_context(tc.tile_pool(name="psA_bc", bufs=1, space="PSUM"))
                ps_ev = pA.enter_context(tc.tile_pool(name="psA_ev", bufs=4, space="PSUM"))
                ps_o = pA.enter_context(tc.tile_pool(name="psA_o", bufs=2, space="PSUM"))

                # [128, 2*CB, C]: blocks 0..7 = diag(tm)W, 8..15 = diag(1-tm)W
                wk_t = wp.tile([128, 2 * CB, C], f8, tag="wk")
                wv_t = wp.tile([128, 2 * CB, C], f8, tag="wv")
                wr_t = wp.tile([128, 2 * CB, C], f8, tag="wr")
                wo_t = wp.tile([128, CB, C], f8, tag="wo")
                ldw(wk_t, Wk, eng=nc.scalar)
                ldw(wv_t, Wv, eng=nc.scalar)
                ldw(wr_t, Wr, eng=nc.scalar)
                ldw(wo_t, Wo, parts=2, eng=nc.scalar)

                ln = {}     # stage-1 products per chunk
                st2a = {}   # stage-2a products (ek/ekv/th) per chunk
                st2 = {}    # stage-2b products per chunk
                for it in range(NCH + 2):
                    # ---- stage 1a: load + square for chunk `it` ----
                    if it < NCH:
                        i = it
                        t0 = i * TC
                        x_b = dbl.tile([128, CB, TC], bf16, tag="xb")
                        nc.sync.dma_start(
                            out=x_b, in_=xTb.rearrange("(cb p) t -> p cb t", p=128)[:, :, t0:t0 + TC])
                        sq = sgf.tile([128, CB, TC], bf16, tag="sq")
                        nc.gpsimd.tensor_mul(sq, x_b, x_b)

                    # ---- stage 3: Wo + residual for chunk `it-3` ----
                    if it >= 3:
                        c2 = it - 3
                        yp = st2[c2]["y8"]
                        x_f = sgf.tile([128, CB, TC], f32, tag="xf")
                        nc.sync.dma_start(
                            out=x_f, in_=xTf.rearrange("(cb p) t -> p cb t", p=128)[
                                :, :, c2 * TC:(c2 + 1) * TC])
                        x2_t = dbl.tile([128, CB, TC], f32r, tag="x2")
                        for co in range(0, CB, 2):
                            ps = ps_o.tile([128, 2, TC], f32, tag="evo")
                            for h in range(2):
                                csl = slice((co + h) * 128, (co + h) * 128 + 128)
                                dr_group(ps[:, h, :], wo_t, yp, csl)
                            nc.vector.scalar_tensor_tensor(
                                out=x2_t[:, co:co + 2, :], in0=ps,
                                scalar=invS_o[:, :], in1=x_f[:, co:co + 2, :],
                                op0=OP.mult, op1=OP.add)
                        nc.scalar.dma_start(out=x2d[c2], in_=x2_t)
                        del st2[c2]["y8"]

                    # ---- stage 2a: time-mix matmuls for chunk `it-1` ----
                    if 1 <= it <= NCH:
                        c = it - 1
                        s1 = ln[c]["s1"]
                        s0 = ln[c]["s0"]
                        ek = dbl.tile([128, CB, TC], bf16, tag="ek")
                        v_t = dbl.tile([128, CB, TC], bf16, tag="v")
                        th_t = dbl.tile([128, CB, TC], bf16, tag="th")
                        for out_t_, w_t_, fn_, sc_, brow in (
                                (ek, wk_t, AF.Exp, sK, BK),
                                (v_t, wv_t, AF.Identity, sV, BV),
                                (th_t, wr_t, AF.Tanh, sR, BR)):
                            for co in range(0, CB, 2):
                                ps = ps_ev.tile([128, 2, TC], f32, tag="ev")
                                for h in range(2):
                                    csl = slice((co + h) * 128, (co + h) * 128 + 128)
                                    dr_group(ps[:, h, :], w_t_[:, 0:CB, :], s1, csl,
                                             stop=False)
                                    dr_group(ps[:, h, :], w_t_[:, CB:2 * CB, :], s0,
                                             csl, start=False)
                                for h in range(2):
                                    nc.scalar.activation(
                                        out=out_t_[:, co + h, :], in_=ps[:, h, :],
                                        func=fn_, scale=sc_,
                                        bias=cv[:, co + h, brow:brow + 1])

                        ekv = dbl.tile([128, CB, TC], bf16, tag="ekv")
                        nc.gpsimd.tensor_mul(ekv, ek, v_t)
                        eum = sgA.tile([128, CB, TC], bf16, tag="eum")
                        nc.gpsimd.tensor_mul(eum, ekv, eu_full)
                        eud = sgA.tile([128, CB, TC], bf16, tag="eud")
                        nc.gpsimd.tensor_mul(eud, ek, eu_full)
                        st2a[c] = {"ek": ek, "ekv": ekv, "th": th_t,
                                   "eum": eum, "eud": eud}
                        if c >= 1:
                            del ln[c - 1]

                    # ---- stage 2b: WKV scan chain for chunk `it-2` ----
                    if 2 <= it <= NCH + 1:
                        c = it - 2
                        ek = st2a[c]["ek"]
                        ekv = st2a[c]["ekv"]
                        th_t = st2a[c]["th"]
                        eum = st2a[c]["eum"]
                        eud = st2a[c]["eud"]

                        A_t = dbl.tile([128, CB, TC + 1], f32, tag="A")
                        B_t = dbl.tile([128, CB, TC + 1], f32, tag="B")
                        if c == 0:
                            nc.vector.memset(A_t[:, :, 0:1], 0.0)
                            nc.gpsimd.memset(B_t[:, :, 0:1], 0.0)
                        else:
                            nc.vector.tensor_copy(out=A_t[:, :, 0:1],
                                                  in_=st2[c - 1]["A"][:, :, TC:TC + 1])
                            nc.gpsimd.tensor_copy(out=B_t[:, :, 0:1],
                                                  in_=st2[c - 1]["B"][:, :, TC:TC + 1])
                        for cb in range(CB):
                            ew_b = _bcast_free(cv[:, cb, EW:EW + 1], TC)
                            nc.vector.tensor_tensor_scan(
                                out=A_t[:, cb, 1:], data0=ew_b, data1=ekv[:, cb, :],
                                initial=A_t[:, cb, 0:1], op0=OP.mult, op1=OP.add)
                            nc.vector.tensor_tensor_scan(
                                out=B_t[:, cb, 1:], data0=ew_b, data1=ek[:, cb, :],
                                initial=B_t[:, cb, 0:1], op0=OP.mult, op1=OP.add)
                        nc.vector.tensor_add(A_t[:, :, 0:TC], A_t[:, :, 0:TC], eum)
                        nc.vector.tensor_add(B_t[:, :, 0:TC], B_t[:, :, 0:TC], eud)
                        nc.vector.reciprocal_approx_fast(out=B_t[:, :, 0:TC],
                                                         in_=B_t[:, :, 0:TC])
                        nc.gpsimd.tensor_mul(B_t[:, :, 0:TC], A_t[:, :, 0:TC],
                                             B_t[:, :, 0:TC])
                        y8 = dbl.tile([128, CB, TC], f8, tag="y8")
                        nc.vector.scalar_tensor_tensor(
                            out=y8, in0=th_t, scalar=one_row[:, :],
                            in1=B_t[:, :, 0:TC], op0=OP.add, op1=OP.mult)
                        st2[c] = {"A": A_t, "B": B_t, "y8": y8}
                        del st2a[c]

                    # ---- stage 1b: LN + s for chunk `it` (emitted after the
                    # stage-2 evacuations: ACT/DVE are strict FIFO, so putting
                    # the stats-dependent ops first would stall the evac flow)
                    if it < NCH:
                        i = it
                        rows = sc1.tile([1, 2, TC], bf16, tag="rows")
                        rtmp = sc1.tile([1, 2, TC], f32, tag="rtmp")
                        mrb = sc1.tile([128, 2, TC], bf16, tag="mrb")
                        layer_norm(ps_stat, ps_bc, rows, rtmp, mrb, x_b, sq)
                        s1 = dbl.tile([128, CB, TC], f8, tag="s1")
                        nc.vector.tensor_mul(s1, x_b,
                                             _bcast_mid(mrb[:, 0, :], CB))
                        nc.vector.tensor_add(s1, s1,
                                             _bcast_mid(mrb[:, 1, :], CB))
                        ln[i] = {"s1": s1}

                    # s0 for chunk `it` (emitted late so it doesn't block the
                    # stage-2 PSUM evacuations on ACT)
                    if it < NCH:
                        s1 = ln[i]["s1"]
                        s0 = dbl.tile([128, CB, TC], f8, tag="s0")
                        if i == 0:
                            nc.vector.memset(s0[:, :, 0:1], 0.0)
                        else:
                            nc.vector.tensor_copy(out=s0[:, :, 0:1],
                                                  in_=ln[i - 1]["s1"][:, :, TC - 1:TC])
                        nc.scalar.activation(out=s0[:, :, 1:], in_=s1[:, :, 0:TC - 1],
                                             func=AF.Copy)
                        ln[i]["s0"] = s0

                # drain: Wo for the last chunk
                yp = st2[NCH - 1]["y8"]
                x_f = sgf.tile([128, CB, TC], f32, tag="xf")
                nc.sync.dma_start(
                    out=x_f, in_=xTf.rearrange("(cb p) t -> p cb t", p=128)[
                        :, :, (NCH - 1) * TC:NCH * TC])
                x2_t = dbl.tile([128, CB, TC], f32r, tag="x2")
                for co in range(0, CB, 2):
                    ps = ps_o.tile([128, 2, TC], f32, tag="evo")
                    for h in range(2):
                        csl = slice((co + h) * 128, (co + h) * 128 + 128)
                        dr_group(ps[:, h, :], wo_t, yp, csl)
                    nc.vector.scalar_tensor_tensor(
                        out=x2_t[:, co:co + 2, :], in0=ps,
                        scalar=invS_o[:, :], in1=x_f[:, co:co + 2, :],
                        op0=OP.mult, op1=OP.add)
                nc.scalar.dma_start(out=x2d[NCH - 1], in_=x2_t)

            # ================= Pass B: FFN =================
            with contextlib.ExitStack() as pB:
                wp = pB.enter_context(tc.tile_pool(name="wB", bufs=1))
                dbl = pB.enter_context(tc.tile_pool(name="dB", bufs=2))
                dbl3 = pB.enter_context(tc.tile_pool(name="dB3", bufs=3))
                sgl = pB.enter_context(tc.tile_pool(name="sB", bufs=1))
                sc1 = pB.enter_context(tc.tile_pool(name="scB", bufs=1))
                rtp = pB.enter_context(tc.tile_pool(name="rtB", bufs=6))
                ps_stat = pB.enter_context(tc.tile_pool(name="psB_st", bufs=1, space="PSUM"))
                ps_bc = pB.enter_context(tc.tile_pool(name="psB_bc", bufs=1, space="PSUM"))
                ps_ev = pB.enter_context(tc.tile_pool(name="psB_ev", bufs=6, space="PSUM"))

                fwk_t = wp.tile([128, CB, 4 * C], fwk_dt, tag="fwk")
                fwv_t = wp.tile([128, FB, C], f8, tag="fwv")
                ldw_m(fwk_t, fWk, mparts=8, eng=nc.gpsimd)
                ldw_m(fwv_t, fWv, mparts=8, eng=nc.scalar)

                ln = {}
                st2 = {}
                for it in range(NCH + 1):
                    # ---- stage 1a: load + square for chunk `it` ----
                    if it < NCH:
                        j = it
                        x2f = dbl3.tile([128, CB, TC], f32r, tag="x2f")
                        nc.sync.dma_start(out=x2f, in_=x2d[j])
                        sq = sgl.tile([128, CB, TC], bf16, tag="sqB")
                        nc.gpsimd.tensor_mul(sq, x2f.bitcast(f32),
                                             x2f.bitcast(f32))

                    # ---- stage 3: fWv + output for chunk `it-2` ----
                    if it >= 2:
                        c2 = it - 2
                        kkp, thp = st2[c2]["kk"], st2[c2]["th2"]
                        xp = ln[c2]["x2f"]
                        ffn = sgl.tile([128, CB, TC], bf16, tag="ffn")
                        for co in range(0, CB, 2):
                            ps = ps_ev.tile([128, 2, TC], f32, tag="evB")
                            for h in range(2):
                                csl = slice((co + h) * 128, (co + h) * 128 + 128)
                                dr_group(ps[:, h, :], fwv_t, kkp, csl)
                            nc.scalar.activation(out=ffn[:, co:co + 2, :], in_=ps,
                                                 func=AF.Identity, scale=sFV)
                        nc.vector.scalar_tensor_tensor(
                            out=ffn, in0=thp, scalar=one_row[:, :], in1=ffn,
                            op0=OP.add, op1=OP.mult)
                        out_t = sgl.tile([128, CB, TC], f32, tag="out")
                        nc.vector.tensor_add(out_t, xp.bitcast(f32), ffn)
                        nc.scalar.dma_start(
                            out=outT.rearrange("(cb p) t -> p cb t", p=128)[
                                :, :, c2 * TC:(c2 + 1) * TC],
                            in_=out_t)
                        del st2[c2], ln[c2]

                    # ---- stage 2: fWk/relu/kk + fWr/tanh for chunk `it-1` ----
                    if 1 <= it <= NCH:
                        c = it - 1
                        fink, finr = ln[c]["fink"], ln[c]["finr"]
                        kk = dbl.tile([128, FB, TC], f8, tag="kk")
                        for co in range(0, FB, 2):
                            ps = ps_ev.tile([128, 2, TC], f32, tag="evB")
                            for h in range(2):
                                csl = slice((co + h) * 128, (co + h) * 128 + 128)
                                if FWK_F8:
                                    dr_group(ps[:, h, :], fwk_t, fink, csl)
                                else:
                                    bf_group(ps[:, h, :], fwk_t, fink, csl)
                            for h in range(2):
                                rt = rtp.tile([128, TC], bf16, tag="rt")
                                if (co + h) % 4 == 3:
                                    nc.vector.scalar_tensor_tensor(
                                        out=rt, in0=ps[:, h, :],
                                        scalar=fbk_t[:, co + h, :],
                                        in1=zeros_b, op0=OP.add, op1=OP.max)
                                else:
                                    nc.scalar.activation(out=rt, in_=ps[:, h, :],
                                                         func=AF.Relu, scale=sFK,
                                                         bias=fbk_t[:, co + h, :])
                                eng = nc.gpsimd if (co + h) % 4 == 3 else nc.vector
                                eng.tensor_mul(kk[:, co + h, :], rt, rt)
                        th2 = dbl.tile([128, CB, TC], bf16, tag="th2")
                        for co in range(0, CB, 2):
                            ps = ps_ev.tile([128, 2, TC], f32, tag="evB")
                            for h in range(2):
                                csl = slice((co + h) * 128, (co + h) * 128 + 128)
                                dr_group(ps[:, h, :], fwr_t, finr, csl)
                            for h in range(2):
                                nc.scalar.activation(out=th2[:, co + h, :],
                                                     in_=ps[:, h, :],
                                                     func=AF.Tanh, scale=sFR,
                                                     bias=cv[:, co + h, BFR:BFR + 1])
                        st2[c] = {"kk": kk, "th2": th2}

                    # ---- stage 1b: LN2 + mixes for chunk `it` (after evacs) ----
                    if it < NCH:
                        rows = sc1.tile([1, 2, TC], bf16, tag="rowsB")
                        rtmp = sc1.tile([1, 2, TC], f32, tag="rtmpB")
                        mrb = sc1.tile([128, 2, TC], bf16, tag="mrbB")
                        layer_norm(ps_stat, ps_bc, rows, rtmp, mrb, x2f, sq)
                        s = dbl.tile([128, CB, TC + 1], bf16, tag="sB")
                        if j == 0:
                            nc.vector.memset(s[:, :, 0:1], 0.0)
                        else:
                            nc.vector.tensor_copy(out=s[:, :, 0:1],
                                                  in_=ln[j - 1]["s"][:, :, TC:TC + 1])
                        nc.vector.tensor_mul(s[:, :, 1:], x2f.bitcast(f32),
                                             _bcast_mid(mrb[:, 0, :], CB))
                        nc.vector.tensor_add(s[:, :, 1:], s[:, :, 1:],
                                             _bcast_mid(mrb[:, 1, :], CB))
                        d = sgl.tile([128, CB, TC], bf16, tag="dB")
                        nc.vector.tensor_sub(d, s[:, :, 1:], s[:, :, 0:TC])
                        fink = dbl.tile([128, CB, TC], fwk_dt, tag="fink")
                        for cb in range(CB):
                            nc.vector.scalar_tensor_tensor(
                                out=fink[:, cb, :], in0=d[:, cb, :],
                                scalar=cv[:, cb, FTMK:FTMK + 1],
                                in1=s[:, cb, 0:TC], op0=OP.mult, op1=OP.add)
                        finr = dbl.tile([128, CB, TC], f8, tag="finr")
                        for cb in range(CB):
                            nc.vector.scalar_tensor_tensor(
                                out=finr[:, cb, :], in0=d[:, cb, :],
                                scalar=cv[:, cb, FTMR:FTMR + 1],
                                in1=s[:, cb, 0:TC], op0=OP.mult, op1=OP.add)
                        ln[j] = {"s": s, "fink": fink, "finr": finr, "x2f": x2f}

                # drain last chunk (stage 3 for NCH-1)
                kkp, thp = st2[NCH - 1]["kk"], st2[NCH - 1]["th2"]
                xp = ln[NCH - 1]["x2f"]
                ffn = sgl.tile([128, CB, TC], bf16, tag="ffn")
                for co in range(0, CB, 2):
                    ps = ps_ev.tile([128, 2, TC], f32, tag="evB")
                    for h in range(2):
                        csl = slice((co + h) * 128, (co + h) * 128 + 128)
                        dr_group(ps[:, h, :], fwv_t, kkp, csl)
                    nc.scalar.activation(out=ffn[:, co:co + 2, :], in_=ps,
                                         func=AF.Identity, scale=sFV)
                nc.vector.scalar_tensor_tensor(
                    out=ffn, in0=thp, scalar=one_row[:, :], in1=ffn,
                    op0=OP.add, op1=OP.mult)
                out_t = sgl.tile([128, CB, TC], f32, tag="out")
                nc.vector.tensor_add(out_t, xp.bitcast(f32), ffn)
                nc.scalar.dma_start(
                    out=outT.rearrange("(cb p) t -> p cb t", p=128)[
                        :, :, (NCH - 1) * TC:NCH * TC],
                    in_=out_t)

    nc.finalize()
    return nc


def _prep_maps(inputs):
    x = np.asarray(inputs["x"], np.float32)
    f32 = np.float32
    bf = ml_dtypes.bfloat16
    f8 = ml_dtypes.float8_e4m3
    g1 = np.asarray(inputs["ln1_g"], f32)
    b1 = np.asarray(inputs["ln1_b"], f32)
    g2 = np.asarray(inputs["ln2_g"], f32)
    b2 = np.asarray(inputs["ln2_b"], f32)
    Wk = np.asarray(inputs["Wk"], f32)
    Wv = np.asarray(inputs["Wv"], f32)
    Wr = np.asarray(inputs["Wr"], f32)
    Wo = np.asarray(inputs["Wo"], f32)
    fWk = np.asarray(inputs["fWk"], f32)
    fWv = np.asarray(inputs["fWv"], f32)
    fWr = np.asarray(inputs["fWr"], f32)
    tmk = np.asarray(inputs["tmk"], f32)
    tmv = np.asarray(inputs["tmv"], f32)
    tmr = np.asarray(inputs["tmr"], f32)
    ftmk = np.asarray(inputs["ftmk"], f32)
    ftmr = np.asarray(inputs["ftmr"], f32)

    ew = np.exp(-np.exp(np.asarray(inputs["time_decay"], f32))).astype(f32)
    eu = np.exp(np.asarray(inputs["time_first"], f32)).astype(f32)

    def crow(v):
        return np.asarray(v, f32).reshape(CB, 128).T  # [128, CB]

    # BV/BR/BFR are halved: v carries the 0.5 of the tanh-form sigmoid, and
    # tanh gets scale=0.5 applied to (z + b) as 0.5*z + 0.5*b.
    rows = np.stack([
        crow(ew), crow(eu),
        crow(b1 @ Wk), crow(0.5 * (b1 @ Wv)), crow(0.5 * (b1 @ Wr)),
        crow(ftmk), crow(ftmr), crow(0.5 * (b2 @ fWr)),
    ], axis=-1)  # [128, CB, NROW]

    fbk = (b2 @ fWk).reshape(FB, 128).T.astype(f32)  # [128, FB]
    onepm = np.stack([np.ones(128, bf), -np.ones(128, bf)])

    def dbl_w(W, g, tm):
        """[2, C, C]: [0]=diag(g*tm)W, [1]=diag(g*(1-tm))W, WS-scaled fp8."""
        base = g[:, None] * W * WS
        return np.stack([tm[:, None] * base,
                         (1.0 - tm)[:, None] * base]).astype(f8)

    common = {
        "cvecs": np.ascontiguousarray(rows),
        "fbk": np.ascontiguousarray(fbk),
        "oneC": np.full(128, 1.0 / C, f32),
        "onePM": onepm,
        "Wk": dbl_w(Wk, g1, tmk),
        "Wv": dbl_w(Wv, g1, tmv),
        "Wr": dbl_w(Wr, g1, tmr),
        "Wo": (WS * Wo).astype(f8),
        "fWk": ((WS if FWK_F8 else 1.0) * g2[:, None] * fWk).astype(
            f8 if FWK_F8 else bf),
        "fWv": (WS * 0.5 * fWv).astype(f8),
        "fWr": (WS * g2[:, None] * fWr).astype(f8),
    }
    maps = []
    for b in range(B):
        xT = np.ascontiguousarray(x[b].T)
        maps.append({**common, "xTf": xT, "xTb": xT.astype(bf)})
    return maps


def get_nc():
    if "nc" not in _CACHE:
        _CACHE["nc"] = _build()
    return _CACHE["nc"]


def kernel(**inputs):
    from concourse.bass_utils import run_bass_kernel_spmd
    nc = get_nc()
    in_maps = _prep_maps(inputs)
    res = run_bass_kernel_spmd(nc, in_maps, core_ids=list(range(B)))
    return np.stack([np.ascontiguousarray(r["outT"].T) for r in res.results])
